# revision 3
# baseline (speedup 1.0000x reference)
"""Trainium2 Bass kernel for nn_Attention_4_lora (B=8, T=1024, C=1024, R=64).

Strategy: data-parallel over the batch dim (1 batch per NeuronCore, 8 cores).
All activations live in transposed [channel, token] layout so that every
matmul contraction runs over the SBUF partition axis. BatchNorm statistics
are reduced across cores with two small AllReduces. The whole datapath is
bf16 (full PE rate at any free size, half the SBUF/DMA of f32r) with exact
f32 PSUM accumulation; tolerance is 2e-2 so ~0.1% element noise is fine.

Per-core pipeline:
  P0  Wmp^T = Wp^T + (Ap@Bp)^T merge; sharded Wc^T = Wp^T @ Wmp^T slice
      (128 c-rows per core via a per-core W_proj column-slice input),
      AllGather of Wc^T (bf16, 2MB) overlapped with all of QKV+attention
  P1  merge Wm_attn^T = W_attn^T + reshape(A@B)^T on device, in d-quarters
      (the torch .view row-major reshape interleaves the LoRA delta with
      stride 3 in the transposed layout; handled with strided SBUF views)
  P2  xa^T[d, t] = Wm^T-slab.T @ x^T  for q,k channels + bn_stats per tile
  P3  v[t, c] (natural layout, needed as AV stationary) + ones-matmul stats
  P4  AllReduce (sum over cores of per-channel mean/E[x^2]) -> normalize
  P5  scores^T[s, t] = k^T-slab.T @ q^T in 256-token chunks (finer causal
      skipping), exp((q.k)/32) on ScalarE, causal mask via affine_select,
      row-sums via ones-matmul
  P6  y^T[c, t] = v-slab.T @ att_exp^T, fused 1/r + v-BN on PSUM drain
  P7  single projection y2^T = Wc^T-slab.T @ y^T -> out [C, T]
      (replaces the double projection: Wc = Wmp @ Wp precomputed sharded)

kernel() takes the full unsharded inputs, shards/uploads, runs SPMD on
cores 0-7, gathers, and transposes back to [B, T, C].
"""

import ml_dtypes
import numpy as np

import concourse.bass as bass
import concourse.mybir as mybir
import concourse.tile as tile
from concourse import bacc
from concourse.bass_utils import run_bass_kernel_spmd

NCORES = 8
C = 1024
R = 64
D3 = 3 * C
EPS = 1e-5
F32 = mybir.dt.float32
BF16 = mybir.dt.bfloat16
AX = mybir.AxisListType
OP = mybir.AluOpType
ACTF = mybir.ActivationFunctionType


def _erange(f, d0, d1):
    """e-range such that d = 3e + f lies in [d0, d1)."""
    el = -((-(d0 - f)) // 3)
    eh = -((-(d1 - f)) // 3)
    return el, eh


def build(T=1024, single_core=False, no_collective=False, reps=1):
    NT = T // 128          # 128-token tiles
    assert T % 512 == 0

    nc = bacc.Bacc(None, target_bir_lowering=False,
                   num_devices=(1 if single_core else NCORES))

    prm = {}
    prm["xT"] = nc.declare_dram_parameter("xT", [C, T], BF16, isOutput=False)
    prm["wT"] = nc.declare_dram_parameter("wT", [C, D3], BF16, isOutput=False)
    prm["wpT"] = nc.declare_dram_parameter("wpT", [C, C], BF16, isOutput=False)
    prm["wnat"] = nc.declare_dram_parameter("wnat", [C, 128], BF16, isOutput=False)
    prm["laT"] = nc.declare_dram_parameter("laT", [R, C], BF16, isOutput=False)
    prm["lbB"] = nc.declare_dram_parameter("lbB", [R, D3], BF16, isOutput=False)
    prm["lpaT"] = nc.declare_dram_parameter("lpaT", [R, C], BF16, isOutput=False)
    prm["lpbB"] = nc.declare_dram_parameter("lpbB", [R, C], BF16, isOutput=False)
    prm["gam"] = nc.declare_dram_parameter("gam", [D3], F32, isOutput=False)
    prm["bet"] = nc.declare_dram_parameter("bet", [D3], F32, isOutput=False)
    prm["out"] = nc.declare_dram_parameter("out", [C, T], F32, isOutput=True)

    with tile.TileContext(nc) as tc:
        for rep in range(reps):
            _emit(nc, tc, prm, T, rep, single_core, no_collective)

    nc.compile()
    return nc


def _emit(nc, tc, prm, T, rep, single_core, no_collective):
    NT = T // 128
    TQ = T // 512          # 512-token chunks (qkv + projection)
    TA = T // 256          # 256-token chunks (attention)
    xT, wT, wpT, laT, lbB = prm["xT"], prm["wT"], prm["wpT"], prm["laT"], prm["lbB"]
    lpaT, lpbB, gam, bet, out = prm["lpaT"], prm["lpbB"], prm["gam"], prm["bet"], prm["out"]
    wnat = prm["wnat"]

    stats_in = nc.dram_tensor(f"stats_in_{rep}", [4096], F32)
    stats_out = nc.dram_tensor(f"stats_out_{rep}", [4096], F32)
    vstats_in = nc.dram_tensor(f"vstats_in_{rep}", [2 * C], F32)
    vstats_out = nc.dram_tensor(f"vstats_out_{rep}", [2 * C], F32)
    rb_dram = nc.dram_tensor(f"rb_{rep}", [T], F32)
    wc_in = nc.dram_tensor(f"wc_in_{rep}", [128 * C], BF16)
    wc_out = nc.dram_tensor(f"wc_out_{rep}", [NCORES * 128 * C], BF16)

    def bcast_dram(param, offset, n):
        return bass.AP(tensor=param[:].tensor, offset=offset, ap=[[0, 128], [1, n]])

    with (
        tc.tile_pool(name=f"misc{rep}", bufs=1) as misc,
        tc.tile_pool(name=f"outst{rep}", bufs=2) as outst,
        tc.tile_pool(name=f"vpool{rep}", bufs=1) as vpool,
        tc.tile_pool(name=f"attp{rep}", bufs=1) as attp,
        tc.tile_pool(name=f"psA{rep}", bufs=4, space="PSUM") as psA,
    ):
        # ---------------- constants / small loads ----------------
        ones_f = misc.tile([128, 1], F32)
        nc.vector.memset(ones_f[:, :], 1.0)
        ones_b = misc.tile([128, 1], BF16)
        nc.vector.tensor_copy(out=ones_b[:, :], in_=ones_f[:, :])
        eps_t = misc.tile([128, 1], F32)
        nc.vector.memset(eps_t[:, :], EPS)

        gqk = misc.tile([128, 16], F32)
        nc.sync.dma_start(out=gqk[:, :], in_=gam[0:2048].rearrange("(i p) -> p i", p=128))
        bqk = misc.tile([128, 16], F32)
        nc.sync.dma_start(out=bqk[:, :], in_=bet[0:2048].rearrange("(i p) -> p i", p=128))

        qk_mv = misc.tile([128, 16, 2], F32)
        m16 = misc.tile([128, 16], F32)
        qa = misc.tile([128, 16], F32)
        qb = misc.tile([128, 16], F32)
        r_bc = misc.tile([128, T], F32)

        xa = [None] * 16
        vnat = [None] * NT

        # ---------------- P0: Wmp merge + sharded Wc^T slice ----------------
        # Wc = Wmp @ Wp; y2^T = Wc^T-slab.T @ y^T. Each core computes
        # Wc^T[128k:128(k+1), :] using its per-core wnat = W_proj[:, 128k:...]
        # input slice, then one AllGather (overlapped with QKV+attention).
        with tc.tile_pool(name=f"wmpp{rep}", bufs=1) as wmppool:
            with tc.tile_pool(name=f"lorap2{rep}", bufs=1) as lorap2:
                lpa_sb = lorap2.tile([R, C], BF16)
                nc.sync.dma_start(out=lpa_sb[:, :], in_=lpaT[:, :])
                lpb_sb = lorap2.tile([R, C], BF16)
                nc.sync.dma_start(out=lpb_sb[:, :], in_=lpbB[:, :])

                wmp = []
                for et in range(8):
                    w2 = wmppool.tile([128, C], BF16, tag=f"wmp{et}", name=f"wmp{et}")
                    nc.sync.dma_start(out=w2[:, :], in_=wpT[128 * et:128 * (et + 1), :])
                    wmp.append(w2)
                wn = []
                for et in range(8):
                    w3 = wmppool.tile([128, 128], BF16, tag=f"wn{et}", name=f"wn{et}")
                    nc.sync.dma_start(out=w3[:, :], in_=wnat[128 * et:128 * (et + 1), :])
                    wn.append(w3)
                for et in range(8):
                    for fc in range(2):
                        ps = psA.tile([128, 512], F32, tag="mm", name=f"dpps{et}_{fc}")
                        nc.tensor.matmul(
                            ps[:, :],
                            lpb_sb[:, 128 * et:128 * (et + 1)],
                            lpa_sb[:, 512 * fc:512 * (fc + 1)],
                            start=True, stop=True)
                        nc.vector.tensor_tensor(
                            out=wmp[et][:, 512 * fc:512 * (fc + 1)],
                            in0=wmp[et][:, 512 * fc:512 * (fc + 1)],
                            in1=ps[:, :], op=OP.add)

            wcsl = misc.tile([128, C], BF16)
            for fc in range(2):
                ps = psA.tile([128, 512], F32, tag="mm", name=f"wcps{fc}")
                for et in range(8):
                    nc.tensor.matmul(
                        ps[:, :],
                        wn[et][:, :],
                        wmp[et][:, 512 * fc:512 * (fc + 1)],
                        start=(et == 0), stop=(et == 7))
                nc.scalar.copy(out=wcsl[:, 512 * fc:512 * (fc + 1)], in_=ps[:, :])
            nc.sync.dma_start(
                out=wc_in[:].rearrange("(p i) -> p i", p=128), in_=wcsl[:, :])
            if single_core or no_collective:
                for k in range(NCORES):
                    nc.sync.dma_start(
                        out=wc_out[128 * C * k:128 * C * (k + 1)], in_=wc_in[:])
            else:
                nc.gpsimd.collective_compute(
                    "AllGather", OP.bypass,
                    replica_groups=[list(range(NCORES))],
                    ins=[wc_in[:]], outs=[wc_out[:]])

        with tc.tile_pool(name=f"xapool{rep}", bufs=1) as xapool:
            with tc.tile_pool(name=f"lorap{rep}", bufs=1) as lorap:
                la_sb = lorap.tile([R, C], BF16)
                nc.sync.dma_start(out=la_sb[:, :], in_=laT[:, :])
                lb_sb = lorap.tile([R, D3], BF16)
                for _c in range(3):
                    nc.sync.dma_start(out=lb_sb[:, 1024 * _c:1024 * (_c + 1)],
                                      in_=lbB[:, 1024 * _c:1024 * (_c + 1)])

                with tc.tile_pool(name=f"xtpool{rep}", bufs=1) as xtpool:
                    with tc.tile_pool(name=f"wb{rep}", bufs=1) as wbp:
                        # ---------------- P1+P2: q,k weight quarters + xa pass
                        # ---------------- then P3: v quarters + natural-v pass
                        bnstat = None

                        def merge_quarter(d0):
                            """Merged Wm^T[:, d0:d0+512] as 8 c-tiles [128, 516]."""
                            wq = []
                            for ct in range(8):
                                w_t = wbp.tile([128, 516], BF16, tag=f"wb{ct}", bufs=1,
                                               name=f"wq{d0}_{ct}")
                                nc.sync.dma_start(
                                    out=w_t[:, 0:512],
                                    in_=wT[128 * ct:128 * (ct + 1), d0:d0 + 512])
                                view3 = w_t[:, :].rearrange("p (u three) -> p u three", three=3)
                                for f in range(3):
                                    el, eh = _erange(f, d0, d0 + 512)
                                    cnt = eh - el
                                    c0 = 3 * el + f - d0
                                    cnt_mm = cnt + (cnt % 2)
                                    es, off = el, 0
                                    if es + cnt_mm > C:
                                        es, off = el - 1, 1
                                    ps = psA.tile([128, 512], F32, tag="mm", name=f"dps{d0}_{ct}_{f}")
                                    nc.tensor.matmul(
                                        ps[:, 0:cnt_mm],
                                        lb_sb[:, 1024 * f + 128 * ct:1024 * f + 128 * (ct + 1)],
                                        la_sb[:, es:es + cnt_mm],
                                        start=True, stop=True)
                                    nc.vector.tensor_tensor(
                                        out=view3[:, 0:cnt, c0],
                                        in0=view3[:, 0:cnt, c0],
                                        in1=ps[:, off:off + cnt], op=OP.add)
                                wq.append(w_t)
                            return wq

                        wq0 = merge_quarter(0)
                        xt = []
                        for k in range(8):
                            x_t = xtpool.tile([128, T], BF16, tag=f"xt{k}", name=f"xt{k}")
                            nc.sync.dma_start(out=x_t[:, :], in_=xT[128 * k:128 * (k + 1), :])
                            xt.append(x_t)

                        for Q in range(4):           # q,k channels: d in [512Q, 512Q+512)
                            wq = wq0 if Q == 0 else merge_quarter(512 * Q)
                            for il in range(4):
                                g = 4 * Q + il
                                xa_g = xapool.tile([128, T], BF16, tag=f"xa{g}",
                                                   name=f"xa{g}")
                                for tch in range(TQ):
                                    ps = psA.tile([128, 512], F32, tag="mm", name=f"xaps{g}_{tch}")
                                    for k in range(8):
                                        nc.tensor.matmul(
                                            ps[:, :],
                                            wq[k][:, 128 * il:128 * (il + 1)],
                                            xt[k][:, 512 * tch:512 * (tch + 1)],
                                            start=(k == 0), stop=(k == 7))
                                    nc.scalar.copy(out=xa_g[:, 512 * tch:512 * (tch + 1)],
                                                   in_=ps[:, :])
                                bnstat = misc.tile([128, TQ, 6], F32, tag="bnstat",
                                                   bufs=2, name=f"bnstat{g}")
                                for j in range(TQ):
                                    nc.vector.bn_stats(out=bnstat[:, j, :],
                                                       in_=xa_g[:, 512 * j:512 * (j + 1)])
                                nc.vector.bn_aggr(out=qk_mv[:, g, :], in_=bnstat[:, :, :])
                                xa[g] = xa_g

                        # qk stats -> (mean, E[x^2]) packed, DMA to stats_in
                        nc.vector.tensor_tensor(out=m16[:, :], in0=qk_mv[:, :, 0],
                                                in1=qk_mv[:, :, 0], op=OP.mult)
                        nc.vector.tensor_tensor(out=qk_mv[:, :, 1], in0=qk_mv[:, :, 1],
                                                in1=m16[:, :], op=OP.add)
                        nc.sync.dma_start(
                            out=stats_in[0:4096].rearrange("(p i s) -> p i s", p=128, s=2),
                            in_=qk_mv[:, :, :])
                        if single_core or no_collective:
                            nc.sync.dma_start(out=stats_out[:], in_=stats_in[:])
                        else:
                            nc.gpsimd.collective_compute(
                                "AllReduce", OP.add,
                                replica_groups=[list(range(NCORES))],
                                ins=[stats_in[:]], outs=[stats_out[:]])
                        ar_qk = misc.tile([128, 16, 2], F32)
                        nc.sync.dma_start(
                            out=ar_qk[:, :, :],
                            in_=stats_out[0:4096].rearrange("(p i s) -> p i s", p=128, s=2))
                        # q,k: a = gamma*rstd, b = beta - mean*a (runs during P3)
                        nc.vector.tensor_scalar(out=ar_qk[:, :, 0], in0=ar_qk[:, :, 0],
                                                scalar1=1.0 / NCORES, scalar2=None, op0=OP.mult)
                        nc.vector.tensor_scalar(out=ar_qk[:, :, 1], in0=ar_qk[:, :, 1],
                                                scalar1=1.0 / NCORES, scalar2=None, op0=OP.mult)
                        nc.vector.tensor_tensor(out=m16[:, :], in0=ar_qk[:, :, 0],
                                                in1=ar_qk[:, :, 0], op=OP.mult)
                        nc.vector.tensor_tensor(out=m16[:, :], in0=ar_qk[:, :, 1],
                                                in1=m16[:, :], op=OP.subtract)
                        nc.scalar.activation(out=m16[:, :], in_=m16[:, :], func=ACTF.Sqrt,
                                             bias=eps_t[:, 0:1])
                        nc.vector.reciprocal(out=m16[:, :], in_=m16[:, :])
                        nc.vector.tensor_tensor(out=qa[:, :], in0=m16[:, :], in1=gqk[:, :],
                                                op=OP.mult)
                        nc.vector.tensor_tensor(out=qb[:, :], in0=ar_qk[:, :, 0], in1=qa[:, :],
                                                op=OP.mult)
                        nc.vector.tensor_tensor(out=qb[:, :], in0=bqk[:, :], in1=qb[:, :],
                                                op=OP.subtract)
                        for g in range(16):
                            nc.vector.tensor_scalar(
                                out=xa[g][:, :], in0=xa[g][:, :],
                                scalar1=qa[:, g:g + 1], scalar2=qb[:, g:g + 1],
                                op0=OP.mult, op1=OP.add)

                        # ---------------- P3: v natural + stats ----------------
                        with tc.tile_pool(name=f"psV{rep}", bufs=1, space="PSUM") as psV:
                            ps_vs = [None, None]
                            ps_vq = [None, None]
                            for Qv in range(2):      # v channels: d in [2048+512Qv, ...)
                                wq = merge_quarter(2048 + 512 * Qv)
                                ps_vs[Qv] = psV.tile([1, 512], F32, tag=f"vs{Qv}",
                                                     name=f"psvs{Qv}")
                                ps_vq[Qv] = psV.tile([1, 512], F32, tag=f"vq{Qv}",
                                                     name=f"psvq{Qv}")
                                for tt in range(NT):
                                    if Qv == 0 and vnat[tt] is None:
                                        vnat[tt] = vpool.tile([128, C], BF16,
                                                              tag=f"v{tt}", name=f"v{tt}")
                                    ps = psA.tile([128, 512], F32, tag="mm", name=f"vps{Qv}_{tt}")
                                    for k in range(8):
                                        nc.tensor.matmul(
                                            ps[:, :],
                                            xt[k][:, 128 * tt:128 * (tt + 1)],
                                            wq[k][:, 0:512],
                                            start=(k == 0), stop=(k == 7))
                                    nc.scalar.copy(
                                        out=vnat[tt][:, 512 * Qv:512 * (Qv + 1)], in_=ps[:, :])
                                    sq = misc.tile([128, 512], BF16, tag="sq", bufs=1,
                                                   name=f"sq{Qv}_{tt}")
                                    nc.scalar.activation(
                                        out=sq[:, :], in_=vnat[tt][:, 512 * Qv:512 * (Qv + 1)],
                                        func=ACTF.Square)
                                    nc.tensor.matmul(ps_vs[Qv][0:1, :], ones_b[:, :],
                                                     vnat[tt][:, 512 * Qv:512 * (Qv + 1)],
                                                     start=(tt == 0), stop=(tt == NT - 1))
                                    nc.tensor.matmul(ps_vq[Qv][0:1, :], ones_b[:, :],
                                                     sq[:, :],
                                                     start=(tt == 0), stop=(tt == NT - 1))
                                vst1 = misc.tile([1, 512], F32, tag="vst", bufs=2,
                                                 name=f"vst1_{Qv}")
                                nc.vector.tensor_copy(out=vst1[0:1, :], in_=ps_vs[Qv][0:1, :])
                                nc.sync.dma_start(
                                    out=vstats_in[512 * Qv:512 * (Qv + 1)], in_=vst1[0:1, :])
                                vst2 = misc.tile([1, 512], F32, tag="vst", bufs=2,
                                                 name=f"vst2_{Qv}")
                                nc.vector.tensor_copy(out=vst2[0:1, :], in_=ps_vq[Qv][0:1, :])
                                nc.sync.dma_start(
                                    out=vstats_in[C + 512 * Qv:C + 512 * (Qv + 1)], in_=vst2[0:1, :])
                            if Qv == 1:
                                if single_core or no_collective:
                                    nc.sync.dma_start(out=vstats_out[:], in_=vstats_in[:])
                                else:
                                    nc.gpsimd.collective_compute(
                                        "AllReduce", OP.add,
                                        replica_groups=[list(range(NCORES))],
                                        ins=[vstats_in[:]], outs=[vstats_out[:]])

            with tc.tile_pool(name=f"bc{rep}", bufs=1) as bcp:
                rstage = bcp.tile([128, T], F32)   # row 0 holds r, then 1/r
                # ------- P5: scores^T (256-chunks), exp, causal, row sums -------
                ae = {}
                scale = 1.0 / float(np.sqrt(C))
                with tc.tile_pool(name=f"psR{rep}", bufs=1, space="PSUM") as psR:
                    for tch in range(TA):
                        acts = [st for st in range(NT) if 128 * st < 256 * (tch + 1)]
                        ps_r = psR.tile([1, 256], F32, tag=f"r{tch}", name=f"psr{tch}")
                        for ii, st in enumerate(acts):
                            ps = psA.tile([128, 512], F32, tag="mm", name=f"scps{tch}_{st}")
                            for j in range(8):
                                nc.tensor.matmul(
                                    ps[:, 0:256],
                                    xa[8 + j][:, 128 * st:128 * (st + 1)],
                                    xa[j][:, 256 * tch:256 * (tch + 1)],
                                    start=(j == 0), stop=(j == 7))
                            a_t = attp.tile([128, 256], BF16, tag=f"ae{tch}_{st}",
                                            name=f"ae{tch}_{st}")
                            nc.scalar.activation(out=a_t[:, :], in_=ps[:, 0:256],
                                                 func=ACTF.Exp, scale=scale)
                            base = 256 * tch - 128 * st
                            if base < 127:
                                nc.gpsimd.affine_select(
                                    out=a_t[:, :], in_=a_t[:, :],
                                    pattern=[[1, 256]], base=base,
                                    channel_multiplier=-1,
                                    compare_op=OP.is_ge, fill=0.0)
                            nc.tensor.matmul(ps_r[0:1, :], ones_b[:, :], a_t[:, :],
                                             start=(ii == 0), stop=(ii == len(acts) - 1))
                            ae[(tch, st)] = a_t
                        nc.vector.tensor_copy(out=rstage[0:1, 256 * tch:256 * (tch + 1)],
                                              in_=ps_r[0:1, :])
                    nc.vector.reciprocal(out=rstage[0:1, :], in_=rstage[0:1, :])
                    nc.sync.dma_start(out=rb_dram[:], in_=rstage[0:1, :])
                    nc.sync.dma_start(out=r_bc[:, :], in_=bcast_dram(rb_dram, 0, T))

            # ---------------- v scale/bias math (readback emitted post-P5) ----
            # y_final = (att_exp @ v_raw) * scale_v / r + bias_v  (v BN folded into
            # the AV drain; scale/bias are per-partition in the y^T layout)
            gv8 = misc.tile([128, 8], F32)
            nc.sync.dma_start(out=gv8[:, :], in_=gam[2048:3072].rearrange("(i p) -> p i", p=128))
            bv8 = misc.tile([128, 8], F32)
            nc.sync.dma_start(out=bv8[:, :], in_=bet[2048:3072].rearrange("(i p) -> p i", p=128))
            vs_m = misc.tile([128, 8], F32)
            nc.sync.dma_start(out=vs_m[:, :], in_=vstats_out[0:C].rearrange("(i p) -> p i", p=128))
            vs_e = misc.tile([128, 8], F32)
            nc.sync.dma_start(out=vs_e[:, :], in_=vstats_out[C:2 * C].rearrange("(i p) -> p i", p=128))
            m8 = misc.tile([128, 8], F32)
            va = misc.tile([128, 8], F32)
            vb = misc.tile([128, 8], F32)
            inv_n = 1.0 / (NCORES * T)
            nc.vector.tensor_scalar(out=vs_m[:, :], in0=vs_m[:, :],
                                    scalar1=inv_n, scalar2=None, op0=OP.mult)
            nc.vector.tensor_scalar(out=vs_e[:, :], in0=vs_e[:, :],
                                    scalar1=inv_n, scalar2=None, op0=OP.mult)
            nc.vector.tensor_tensor(out=m8[:, :], in0=vs_m[:, :], in1=vs_m[:, :], op=OP.mult)
            nc.vector.tensor_tensor(out=m8[:, :], in0=vs_e[:, :], in1=m8[:, :], op=OP.subtract)
            nc.scalar.activation(out=m8[:, :], in_=m8[:, :], func=ACTF.Sqrt,
                                 bias=eps_t[:, 0:1])
            nc.vector.reciprocal(out=m8[:, :], in_=m8[:, :])
            nc.vector.tensor_tensor(out=va[:, :], in0=m8[:, :], in1=gv8[:, :], op=OP.mult)
            nc.vector.tensor_tensor(out=vb[:, :], in0=vs_m[:, :], in1=va[:, :], op=OP.mult)
            nc.vector.tensor_tensor(out=vb[:, :], in0=bv8[:, :], in1=vb[:, :], op=OP.subtract)

        # xapool closed (frees 32KB/partition for the projection weights)
        with (
            tc.tile_pool(name=f"projp{rep}", bufs=1) as projp,
            tc.tile_pool(name=f"psP{rep}", bufs=2, space="PSUM") as psP,
        ):
            # ---------------- P6: AV (256-chunks) + fused 1/r + v-BN ----------
            y = [None] * 8
            for tch in range(TA):
                acts = [st for st in range(NT) if 128 * st < 256 * (tch + 1)]
                for ct in range(8):
                    ps = psA.tile([128, 512], F32, tag="mm", name=f"avps{tch}_{ct}")
                    for ii, st in enumerate(acts):
                        nc.tensor.matmul(
                            ps[:, 0:256],
                            vnat[st][:, 128 * ct:128 * (ct + 1)],
                            ae[(tch, st)][:, :],
                            start=(ii == 0), stop=(ii == len(acts) - 1))
                    if y[ct] is None:
                        y[ct] = projp.tile([128, T], BF16, tag=f"y{ct}", name=f"y{ct}")
                    ysl = y[ct][:, 256 * tch:256 * (tch + 1)]
                    nc.vector.tensor_tensor(
                        out=ysl, in0=ps[:, 0:256], in1=r_bc[:, 256 * tch:256 * (tch + 1)],
                        op=OP.mult)
                    nc.vector.tensor_scalar(
                        out=ysl, in0=ysl,
                        scalar1=va[:, ct:ct + 1], scalar2=vb[:, ct:ct + 1],
                        op0=OP.mult, op1=OP.add)

            # ---------------- P7: single projection with gathered Wc^T --------
            wc = []
            for ct in range(8):
                w1 = projp.tile([128, C], BF16, tag=f"wc{ct}", name=f"wc{ct}")
                nc.sync.dma_start(
                    out=w1[:, :],
                    in_=wc_out[128 * C * ct:128 * C * (ct + 1)].rearrange(
                        "(p i) -> p i", p=128))
                wc.append(w1)
            for tch in range(TQ):
                for ft in range(8):
                    ps = psP.tile([128, 512], F32, tag="pp", name=f"p2ps{tch}_{ft}")
                    for ct in range(8):
                        nc.tensor.matmul(
                            ps[:, :],
                            wc[ct][:, 128 * ft:128 * (ft + 1)],
                            y[ct][:, 512 * tch:512 * (tch + 1)],
                            start=(ct == 0), stop=(ct == 7))
                    o_t = outst.tile([128, 512], F32, tag="o", name=f"o{tch}_{ft}")
                    nc.vector.tensor_copy(out=o_t[:, :], in_=ps[:, :])
                    nc.sync.dma_start(
                        out=out[128 * ft:128 * (ft + 1), 512 * tch:512 * (tch + 1)],
                        in_=o_t[:, :])


_NC_CACHE = {}


def _get_nc(T):
    if T not in _NC_CACHE:
        _NC_CACHE[T] = build(T)
    return _NC_CACHE[T]


LAST_RESULTS = None
LAST_IN_MAPS = None


def make_in_maps(x, W_attn, W_proj, lora_attn_A, lora_attn_B, lora_proj_A,
                 lora_proj_B, bn_gamma, bn_beta, **_unused):
    f = np.float32
    bf = ml_dtypes.bfloat16
    x = np.asarray(x, f)
    B = x.shape[0]
    wT = np.ascontiguousarray(np.asarray(W_attn, f).T.astype(bf))      # [C, 3C]
    wp = np.asarray(W_proj, f)
    wpT = np.ascontiguousarray(wp.T.astype(bf))                        # [C, C]
    laT = np.ascontiguousarray(np.asarray(lora_attn_A, f).T.astype(bf))   # [R, C]
    lbB = np.ascontiguousarray(np.asarray(lora_attn_B, f).astype(bf))     # [R, 3C]
    lpaT = np.ascontiguousarray(np.asarray(lora_proj_A, f).T.astype(bf))  # [R, C]
    lpbB = np.ascontiguousarray(np.asarray(lora_proj_B, f).astype(bf))    # [R, C]
    gam = np.ascontiguousarray(np.asarray(bn_gamma, f))
    bet = np.ascontiguousarray(np.asarray(bn_beta, f))

    in_maps = []
    for b in range(B):
        in_maps.append({
            "xT": np.ascontiguousarray(x[b].T.astype(bf)),
            "wT": wT, "wpT": wpT,
            "wnat": np.ascontiguousarray(wp[:, 128 * b:128 * (b + 1)].astype(bf)),
            "laT": laT, "lbB": lbB,
            "lpaT": lpaT, "lpbB": lpbB, "gam": gam, "bet": bet,
        })
    return in_maps


def kernel(x, W_attn, W_proj, lora_attn_A, lora_attn_B, lora_proj_A, lora_proj_B,
           bn_gamma, bn_beta):
    global LAST_RESULTS, LAST_IN_MAPS
    f = np.float32
    x = np.asarray(x, f)
    B, T, C_ = x.shape
    assert C_ == C and B == NCORES

    in_maps = make_in_maps(x, W_attn, W_proj, lora_attn_A, lora_attn_B,
                           lora_proj_A, lora_proj_B, bn_gamma, bn_beta)
    LAST_IN_MAPS = in_maps
    nc = _get_nc(T)
    res = run_bass_kernel_spmd(nc, in_maps, core_ids=list(range(NCORES)))
    LAST_RESULTS = res
    return np.stack([np.asarray(res.results[b]["out"]).T for b in range(B)]).astype(f)


# revision 14
# speedup vs baseline: 1.3850x; 1.3850x over previous
"""Trainium2 Bass kernel for nn_Attention_4_lora (B=8, T=1024, C=1024, R=64).

Strategy: data-parallel over the batch dim (1 batch per NeuronCore, 8 cores).
All activations live in transposed [channel, token] layout so that every
matmul contraction runs over the SBUF partition axis. BatchNorm statistics
are reduced across cores with two small AllReduces. The whole datapath is
bf16 (full PE rate at any free size, half the SBUF/DMA of f32r) with exact
f32 PSUM accumulation; tolerance is 2e-2 so ~0.1% element noise is fine.

Per-core pipeline:
  P0  Wmp^T = Wp^T + (Ap@Bp)^T merge; sharded Wc^T = Wp^T @ Wmp^T slice
      (128 c-rows per core via a per-core W_proj column-slice input),
      AllGather of Wc^T (bf16, 2MB) overlapped with all of QKV+attention
  P1  merge Wm_attn^T = W_attn^T + reshape(A@B)^T on device, in d-quarters
      (the torch .view row-major reshape interleaves the LoRA delta with
      stride 3 in the transposed layout; handled with strided SBUF views)
  P2  xa^T[d, t] = Wm^T-slab.T @ x^T  for q,k channels + bn_stats per tile
  P3  v[t, c] (natural layout, needed as AV stationary) + ones-matmul stats
  P4  AllReduce (sum over cores of per-channel mean/E[x^2]) -> normalize
  P5  scores^T[s, t] = k^T-slab.T @ q^T in 256-token chunks (finer causal
      skipping), exp((q.k)/32) on ScalarE, causal mask via affine_select,
      row-sums via ones-matmul
  P6  y^T[c, t] = v-slab.T @ att_exp^T, fused 1/r + v-BN on PSUM drain
  P7  single projection y2^T = Wc^T-slab.T @ y^T -> out [C, T]
      (replaces the double projection: Wc = Wmp @ Wp precomputed sharded)

kernel() takes the full unsharded inputs, shards/uploads, runs SPMD on
cores 0-7, gathers, and transposes back to [B, T, C].
"""

import ml_dtypes
import numpy as np

import concourse.bass as bass
import concourse.mybir as mybir
import concourse.tile as tile
from concourse import bacc
from concourse.bass_utils import run_bass_kernel_spmd

NCORES = 8
C = 1024
R = 64
D3 = 3 * C
EPS = 1e-5
F32 = mybir.dt.float32
BF16 = mybir.dt.bfloat16
AX = mybir.AxisListType
OP = mybir.AluOpType
ACTF = mybir.ActivationFunctionType


def _erange(f, d0, d1):
    """e-range such that d = 3e + f lies in [d0, d1)."""
    el = -((-(d0 - f)) // 3)
    eh = -((-(d1 - f)) // 3)
    return el, eh


def build(T=1024, single_core=False, no_collective=False, reps=1):
    NT = T // 128          # 128-token tiles
    assert T % 512 == 0

    nc = bacc.Bacc(None, target_bir_lowering=False,
                   num_devices=(1 if single_core else NCORES))

    prm = {}
    prm["xT"] = nc.declare_dram_parameter("xT", [C, T], BF16, isOutput=False)
    prm["wT"] = nc.declare_dram_parameter("wT", [C, D3], BF16, isOutput=False)
    prm["wpT"] = nc.declare_dram_parameter("wpT", [C, C], BF16, isOutput=False)
    prm["wnat"] = nc.declare_dram_parameter("wnat", [C, 128], BF16, isOutput=False)
    prm["laT"] = nc.declare_dram_parameter("laT", [R, C], BF16, isOutput=False)
    prm["lbB"] = nc.declare_dram_parameter("lbB", [R, D3], BF16, isOutput=False)
    prm["lpaT"] = nc.declare_dram_parameter("lpaT", [R, C], BF16, isOutput=False)
    prm["lpbB"] = nc.declare_dram_parameter("lpbB", [R, C], BF16, isOutput=False)
    prm["gam"] = nc.declare_dram_parameter("gam", [D3], F32, isOutput=False)
    prm["bet"] = nc.declare_dram_parameter("bet", [D3], F32, isOutput=False)
    prm["out"] = nc.declare_dram_parameter("out", [C, T], F32, isOutput=True)

    with tile.TileContext(nc) as tc:
        for rep in range(reps):
            _emit(nc, tc, prm, T, rep, single_core, no_collective)

    nc.compile()
    return nc


def _emit(nc, tc, prm, T, rep, single_core, no_collective):
    NT = T // 128
    TQ = T // 512          # 512-token chunks (qkv + projection)
    TA = T // 256          # 256-token chunks (attention)
    xT, wT, wpT, laT, lbB = prm["xT"], prm["wT"], prm["wpT"], prm["laT"], prm["lbB"]
    lpaT, lpbB, gam, bet, out = prm["lpaT"], prm["lpbB"], prm["gam"], prm["bet"], prm["out"]
    wnat = prm["wnat"]

    stats_in = nc.dram_tensor(f"stats_in_{rep}", [4096], F32)
    stats_out = nc.dram_tensor(f"stats_out_{rep}", [4096], F32, addr_space="Shared")
    vstats_in = nc.dram_tensor(f"vstats_in_{rep}", [2 * C], F32)
    vstats_out = nc.dram_tensor(f"vstats_out_{rep}", [2 * C], F32, addr_space="Shared")
    rb_dram = nc.dram_tensor(f"rb_{rep}", [T], F32)
    wc_in = nc.dram_tensor(f"wc_in_{rep}", [128 * C], BF16)
    wc_out = nc.dram_tensor(f"wc_out_{rep}", [NCORES * 128 * C], BF16,
                            addr_space="Shared")

    def bcast_dram(param, offset, n):
        return bass.AP(tensor=param[:].tensor, offset=offset, ap=[[0, 128], [1, n]])

    with (
        tc.tile_pool(name=f"misc{rep}", bufs=1) as misc,
        tc.tile_pool(name=f"outst{rep}", bufs=2) as outst,
        tc.tile_pool(name=f"vpool{rep}", bufs=1) as vpool,
        tc.tile_pool(name=f"attp{rep}", bufs=1) as attp,
        tc.tile_pool(name=f"psA{rep}", bufs=4, space="PSUM") as psA,
    ):
        # ---------------- constants / small loads ----------------
        ones_f = misc.tile([128, 1], F32)
        nc.vector.memset(ones_f[:, :], 1.0)
        ones_b = misc.tile([128, 1], BF16)
        nc.vector.tensor_copy(out=ones_b[:, :], in_=ones_f[:, :])
        eps_t = misc.tile([128, 1], F32)
        nc.vector.memset(eps_t[:, :], EPS)

        gqk = misc.tile([128, 16], F32)
        bqk = misc.tile([128, 16], F32)
        qk_mv = misc.tile([128, 16, 2], F32)
        m16 = misc.tile([128, 16], F32)
        qa = misc.tile([128, 16], F32)
        qb = misc.tile([128, 16], F32)
        r_bc = misc.tile([128, T], F32)

        xa = [None] * 16
        vnat = [None] * NT

        # ---------------- P0: Wmp merge + sharded Wc^T slice ----------------
        # Wc = Wmp @ Wp; y2^T = Wc^T-slab.T @ y^T. Each core computes
        # Wc^T[128k:128(k+1), :] using its per-core wnat = W_proj[:, 128k:...]
        # input slice, then one AllGather (overlapped with QKV+attention).
        with tc.tile_pool(name=f"wmpp{rep}", bufs=1) as wmppool:
            with tc.tile_pool(name=f"lorap2{rep}", bufs=1) as lorap2:
                lpa_sb = lorap2.tile([R, C], BF16)
                nc.sync.dma_start(out=lpa_sb[:, :], in_=lpaT[:, :])
                lpb_sb = lorap2.tile([R, C], BF16)
                nc.sync.dma_start(out=lpb_sb[:, :], in_=lpbB[:, :])

                wmp = []
                for et in range(8):
                    w2 = wmppool.tile([128, C], BF16, tag=f"wmp{et}", name=f"wmp{et}")
                    nc.sync.dma_start(out=w2[:, :], in_=wpT[128 * et:128 * (et + 1), :])
                    wmp.append(w2)
                wn = []
                for et in range(8):
                    w3 = wmppool.tile([128, 128], BF16, tag=f"wn{et}", name=f"wn{et}")
                    nc.sync.dma_start(out=w3[:, :], in_=wnat[128 * et:128 * (et + 1), :])
                    wn.append(w3)
                for et in range(8):
                    for fc in range(2):
                        ps = psA.tile([128, 512], F32, tag="mm", name=f"dpps{et}_{fc}")
                        nc.tensor.matmul(
                            ps[:, :],
                            lpb_sb[:, 128 * et:128 * (et + 1)],
                            lpa_sb[:, 512 * fc:512 * (fc + 1)],
                            start=True, stop=True)
                        nc.vector.tensor_tensor(
                            out=wmp[et][:, 512 * fc:512 * (fc + 1)],
                            in0=wmp[et][:, 512 * fc:512 * (fc + 1)],
                            in1=ps[:, :], op=OP.add)

            wcsl = misc.tile([128, C], BF16)
            for fc in range(2):
                ps = psA.tile([128, 512], F32, tag="mm", name=f"wcps{fc}")
                for et in range(8):
                    nc.tensor.matmul(
                        ps[:, :],
                        wn[et][:, :],
                        wmp[et][:, 512 * fc:512 * (fc + 1)],
                        start=(et == 0), stop=(et == 7))
                nc.scalar.copy(out=wcsl[:, 512 * fc:512 * (fc + 1)], in_=ps[:, :])
            nc.sync.dma_start(
                out=wc_in[:].rearrange("(p i) -> p i", p=128), in_=wcsl[:, :])
            if single_core or no_collective:
                for k in range(NCORES):
                    nc.sync.dma_start(
                        out=wc_out[128 * C * k:128 * C * (k + 1)], in_=wc_in[:])
            else:
                nc.gpsimd.collective_compute(
                    "AllGather", OP.bypass,
                    replica_groups=[list(range(NCORES))],
                    ins=[wc_in[:]], outs=[wc_out[:]])

        with tc.tile_pool(name=f"xapool{rep}", bufs=1) as xapool:
            with tc.tile_pool(name=f"lorap{rep}", bufs=1) as lorap:
                la_sb = lorap.tile([R, C], BF16)
                nc.sync.dma_start(out=la_sb[:, :], in_=laT[:, :])
                lb_sb = lorap.tile([R, D3], BF16)
                nc.sync.dma_start(out=lb_sb[:, :], in_=lbB[:, :])

                with tc.tile_pool(name=f"xtpool{rep}", bufs=1) as xtpool:
                    with tc.tile_pool(name=f"wb{rep}", bufs=1) as wbp:
                        # ---------------- P1+P2: q,k weight quarters + xa pass
                        # ---------------- then P3: v quarters + natural-v pass
                        bnstat = None

                        def merge_quarter(d0):
                            """Merged Wm^T[:, d0:d0+512] as 8 c-tiles [128, 516]."""
                            wq = []
                            for ct in range(8):
                                w_t = wbp.tile([128, 516], BF16, tag=f"wb{ct}", bufs=2,
                                               name=f"wq{d0}_{ct}")
                                nc.sync.dma_start(
                                    out=w_t[:, 0:512],
                                    in_=wT[128 * ct:128 * (ct + 1), d0:d0 + 512])
                                view3 = w_t[:, :].rearrange("p (u three) -> p u three", three=3)
                                for f in range(3):
                                    el, eh = _erange(f, d0, d0 + 512)
                                    cnt = eh - el
                                    c0 = 3 * el + f - d0
                                    cnt_mm = cnt + (cnt % 2)
                                    es, off = el, 0
                                    if es + cnt_mm > C:
                                        es, off = el - 1, 1
                                    ps = psA.tile([128, 512], F32, tag="mm", name=f"dps{d0}_{ct}_{f}")
                                    nc.tensor.matmul(
                                        ps[:, 0:cnt_mm],
                                        lb_sb[:, 1024 * f + 128 * ct:1024 * f + 128 * (ct + 1)],
                                        la_sb[:, es:es + cnt_mm],
                                        start=True, stop=True)
                                    nc.vector.tensor_tensor(
                                        out=view3[:, 0:cnt, c0],
                                        in0=view3[:, 0:cnt, c0],
                                        in1=ps[:, off:off + cnt], op=OP.add)
                                wq.append(w_t)
                            return wq

                        wq0 = merge_quarter(0)
                        xt = []
                        for k in range(8):
                            x_t = xtpool.tile([128, T], BF16, tag=f"xt{k}", name=f"xt{k}")
                            nc.sync.dma_start(out=x_t[:, :], in_=xT[128 * k:128 * (k + 1), :])
                            xt.append(x_t)
                        nc.sync.dma_start(out=gqk[:, :],
                                          in_=gam[0:2048].rearrange("(i p) -> p i", p=128))
                        nc.sync.dma_start(out=bqk[:, :],
                                          in_=bet[0:2048].rearrange("(i p) -> p i", p=128))

                        for Q in range(4):           # q,k channels: d in [512Q, 512Q+512)
                            wq = wq0 if Q == 0 else merge_quarter(512 * Q)
                            for il in range(4):
                                g = 4 * Q + il
                                xa_g = xapool.tile([128, T], BF16, tag=f"xa{g}",
                                                   name=f"xa{g}")
                                for tch in range(TQ):
                                    ps = psA.tile([128, 512], F32, tag="mm", name=f"xaps{g}_{tch}")
                                    for k in range(8):
                                        nc.tensor.matmul(
                                            ps[:, :],
                                            wq[k][:, 128 * il:128 * (il + 1)],
                                            xt[k][:, 512 * tch:512 * (tch + 1)],
                                            start=(k == 0), stop=(k == 7))
                                    nc.scalar.copy(out=xa_g[:, 512 * tch:512 * (tch + 1)],
                                                   in_=ps[:, :])
                                bnstat = misc.tile([128, TQ, 6], F32, tag="bnstat",
                                                   bufs=2, name=f"bnstat{g}")
                                for j in range(TQ):
                                    nc.vector.bn_stats(out=bnstat[:, j, :],
                                                       in_=xa_g[:, 512 * j:512 * (j + 1)])
                                nc.vector.bn_aggr(out=qk_mv[:, g, :], in_=bnstat[:, :, :])
                                xa[g] = xa_g

                        # qk stats -> (mean, E[x^2]) packed, DMA to stats_in
                        nc.vector.tensor_tensor(out=m16[:, :], in0=qk_mv[:, :, 0],
                                                in1=qk_mv[:, :, 0], op=OP.mult)
                        nc.vector.tensor_tensor(out=qk_mv[:, :, 1], in0=qk_mv[:, :, 1],
                                                in1=m16[:, :], op=OP.add)
                        nc.sync.dma_start(
                            out=stats_in[0:4096].rearrange("(p i s) -> p i s", p=128, s=2),
                            in_=qk_mv[:, :, :])
                        if single_core or no_collective:
                            nc.sync.dma_start(out=stats_out[:], in_=stats_in[:])
                        else:
                            nc.gpsimd.collective_compute(
                                "AllReduce", OP.add,
                                replica_groups=[list(range(NCORES))],
                                ins=[stats_in[:]], outs=[stats_out[:]])
                        ar_qk = misc.tile([128, 16, 2], F32)
                        nc.sync.dma_start(
                            out=ar_qk[:, :, :],
                            in_=stats_out[0:4096].rearrange("(p i s) -> p i s", p=128, s=2))
                        # q,k: a = gamma*rstd, b = beta - mean*a (runs during P3)
                        nc.vector.tensor_scalar(out=ar_qk[:, :, 0], in0=ar_qk[:, :, 0],
                                                scalar1=1.0 / NCORES, scalar2=None, op0=OP.mult)
                        nc.vector.tensor_scalar(out=ar_qk[:, :, 1], in0=ar_qk[:, :, 1],
                                                scalar1=1.0 / NCORES, scalar2=None, op0=OP.mult)
                        nc.vector.tensor_tensor(out=m16[:, :], in0=ar_qk[:, :, 0],
                                                in1=ar_qk[:, :, 0], op=OP.mult)
                        nc.vector.tensor_tensor(out=m16[:, :], in0=ar_qk[:, :, 1],
                                                in1=m16[:, :], op=OP.subtract)
                        nc.scalar.activation(out=m16[:, :], in_=m16[:, :], func=ACTF.Sqrt,
                                             bias=eps_t[:, 0:1])
                        nc.vector.reciprocal(out=m16[:, :], in_=m16[:, :])
                        nc.vector.tensor_tensor(out=qa[:, :], in0=m16[:, :], in1=gqk[:, :],
                                                op=OP.mult)
                        nc.vector.tensor_tensor(out=qb[:, :], in0=ar_qk[:, :, 0], in1=qa[:, :],
                                                op=OP.mult)
                        nc.vector.tensor_tensor(out=qb[:, :], in0=bqk[:, :], in1=qb[:, :],
                                                op=OP.subtract)
                        # normalize in 256-col chunks, chunk-outer, so P5's
                        # first score chunks can start before the whole pass
                        # finishes
                        for cc in range(4):
                            for g in range(16):
                                nc.vector.tensor_scalar(
                                    out=xa[g][:, 256 * cc:256 * (cc + 1)],
                                    in0=xa[g][:, 256 * cc:256 * (cc + 1)],
                                    scalar1=qa[:, g:g + 1], scalar2=qb[:, g:g + 1],
                                    op0=OP.mult, op1=OP.add)

                        # ---------------- P3: v natural + stats ----------------
                        with tc.tile_pool(name=f"psV{rep}", bufs=1, space="PSUM") as psV:
                            ps_vs = [None, None]
                            ps_vq = [None, None]
                            for Qv in range(2):      # v channels: d in [2048+512Qv, ...)
                                wq = merge_quarter(2048 + 512 * Qv)
                                ps_vs[Qv] = psV.tile([1, 512], F32, tag=f"vs{Qv}",
                                                     name=f"psvs{Qv}")
                                ps_vq[Qv] = psV.tile([1, 512], F32, tag=f"vq{Qv}",
                                                     name=f"psvq{Qv}")
                                for tt in range(NT):
                                    if Qv == 0 and vnat[tt] is None:
                                        vnat[tt] = vpool.tile([128, C], BF16,
                                                              tag=f"v{tt}", name=f"v{tt}")
                                    ps = psA.tile([128, 512], F32, tag="mm", name=f"vps{Qv}_{tt}")
                                    for k in range(8):
                                        nc.tensor.matmul(
                                            ps[:, :],
                                            xt[k][:, 128 * tt:128 * (tt + 1)],
                                            wq[k][:, 0:512],
                                            start=(k == 0), stop=(k == 7))
                                    nc.scalar.copy(
                                        out=vnat[tt][:, 512 * Qv:512 * (Qv + 1)], in_=ps[:, :])
                                    sq = misc.tile([128, 512], BF16, tag="sq", bufs=2,
                                                   name=f"sq{Qv}_{tt}")
                                    nc.gpsimd.tensor_mul(
                                        out=sq[:, :],
                                        in0=vnat[tt][:, 512 * Qv:512 * (Qv + 1)],
                                        in1=vnat[tt][:, 512 * Qv:512 * (Qv + 1)])
                                    nc.tensor.matmul(ps_vs[Qv][0:1, :], ones_b[:, :],
                                                     vnat[tt][:, 512 * Qv:512 * (Qv + 1)],
                                                     start=(tt == 0), stop=(tt == NT - 1))
                                    nc.tensor.matmul(ps_vq[Qv][0:1, :], ones_b[:, :],
                                                     sq[:, :],
                                                     start=(tt == 0), stop=(tt == NT - 1))
                                vst1 = misc.tile([1, 512], F32, tag="vst", bufs=2,
                                                 name=f"vst1_{Qv}")
                                nc.vector.tensor_copy(out=vst1[0:1, :], in_=ps_vs[Qv][0:1, :])
                                nc.sync.dma_start(
                                    out=vstats_in[512 * Qv:512 * (Qv + 1)], in_=vst1[0:1, :])
                                vst2 = misc.tile([1, 512], F32, tag="vst", bufs=2,
                                                 name=f"vst2_{Qv}")
                                nc.vector.tensor_copy(out=vst2[0:1, :], in_=ps_vq[Qv][0:1, :])
                                nc.sync.dma_start(
                                    out=vstats_in[C + 512 * Qv:C + 512 * (Qv + 1)], in_=vst2[0:1, :])
                            if Qv == 1:
                                if single_core or no_collective:
                                    nc.sync.dma_start(out=vstats_out[:], in_=vstats_in[:])
                                else:
                                    nc.gpsimd.collective_compute(
                                        "AllReduce", OP.add,
                                        replica_groups=[list(range(NCORES))],
                                        ins=[vstats_in[:]], outs=[vstats_out[:]])

            with tc.tile_pool(name=f"bc{rep}", bufs=1) as bcp:
                rstage = bcp.tile([128, T], F32)   # row 0 holds r, then 1/r
                # ------- P5: scores^T (256-chunks), exp, causal, row sums -------
                ae = {}
                scale = 1.0 / float(np.sqrt(C))
                with tc.tile_pool(name=f"psR{rep}", bufs=1, space="PSUM") as psR:
                    for tch in range(TA):
                        acts = [st for st in range(NT) if 128 * st < 256 * (tch + 1)]
                        ps_r = psR.tile([1, 256], F32, tag=f"r{tch}", name=f"psr{tch}")
                        for ii, st in enumerate(acts):
                            ps = psA.tile([128, 512], F32, tag="mm", name=f"scps{tch}_{st}")
                            for j in range(8):
                                nc.tensor.matmul(
                                    ps[:, 0:256],
                                    xa[8 + j][:, 128 * st:128 * (st + 1)],
                                    xa[j][:, 256 * tch:256 * (tch + 1)],
                                    start=(j == 0), stop=(j == 7))
                            a_t = attp.tile([128, 256], BF16, tag=f"ae{tch}_{st}",
                                            name=f"ae{tch}_{st}")
                            nc.scalar.activation(out=a_t[:, :], in_=ps[:, 0:256],
                                                 func=ACTF.Exp, scale=scale)
                            base = 256 * tch - 128 * st
                            if base < 127:
                                nc.gpsimd.affine_select(
                                    out=a_t[:, :], in_=a_t[:, :],
                                    pattern=[[1, 256]], base=base,
                                    channel_multiplier=-1,
                                    compare_op=OP.is_ge, fill=0.0)
                            nc.tensor.matmul(ps_r[0:1, :], ones_b[:, :], a_t[:, :],
                                             start=(ii == 0), stop=(ii == len(acts) - 1))
                            ae[(tch, st)] = a_t
                        nc.vector.tensor_copy(out=rstage[0:1, 256 * tch:256 * (tch + 1)],
                                              in_=ps_r[0:1, :])
                    nc.vector.reciprocal(out=rstage[0:1, :], in_=rstage[0:1, :])
                    nc.sync.dma_start(out=rb_dram[:], in_=rstage[0:1, :])
                    nc.sync.dma_start(out=r_bc[:, :], in_=bcast_dram(rb_dram, 0, T))

        # xapool closed (frees 32KB/partition for the projection weights)
        with (
            tc.tile_pool(name=f"projp{rep}", bufs=1) as projp,
            tc.tile_pool(name=f"psP{rep}", bufs=2, space="PSUM") as psP,
        ):
            # ---------------- P6: AV (256-chunks), plain PSUM drain -----------
            # v-BN (va, vb) and 1/r are all folded into P7:
            #   y2[f,t] = (1/r_t) * sum_c (Wc[f,c]*va_c) * AVraw[c,t] + wcvb_f
            # with wcvb_f = sum_c Wc^T[c,f]*vb_c  (since sum_s ae[s,t] = r_t).
            # So AV drains have no dependency on the v-stats AllReduce.
            y = [None] * 8
            for tch in range(TA):
                acts = [st for st in range(NT) if 128 * st < 256 * (tch + 1)]
                for ct in range(8):
                    ps = psA.tile([128, 512], F32, tag="mm", name=f"avps{tch}_{ct}")
                    for ii, st in enumerate(acts):
                        nc.tensor.matmul(
                            ps[:, 0:256],
                            vnat[st][:, 128 * ct:128 * (ct + 1)],
                            ae[(tch, st)][:, :],
                            start=(ii == 0), stop=(ii == len(acts) - 1))
                    if y[ct] is None:
                        y[ct] = projp.tile([128, T], BF16, tag=f"y{ct}", name=f"y{ct}")
                    nc.scalar.copy(out=y[ct][:, 256 * tch:256 * (tch + 1)],
                                   in_=ps[:, 0:256])

            wc = []
            for ct in range(8):
                w1 = projp.tile([128, C], BF16, tag=f"wc{ct}", name=f"wc{ct}")
                nc.sync.dma_start(
                    out=w1[:, :],
                    in_=wc_out[128 * C * ct:128 * C * (ct + 1)].rearrange(
                        "(p i) -> p i", p=128))
                wc.append(w1)

            # ------------- v scale/bias math (gates only wc scaling) ----------
            gv8 = misc.tile([128, 8], F32)
            nc.sync.dma_start(out=gv8[:, :], in_=gam[2048:3072].rearrange("(i p) -> p i", p=128))
            bv8 = misc.tile([128, 8], F32)
            nc.sync.dma_start(out=bv8[:, :], in_=bet[2048:3072].rearrange("(i p) -> p i", p=128))
            vs_m = misc.tile([128, 8], F32)
            nc.sync.dma_start(out=vs_m[:, :], in_=vstats_out[0:C].rearrange("(i p) -> p i", p=128))
            vs_e = misc.tile([128, 8], F32)
            nc.sync.dma_start(out=vs_e[:, :], in_=vstats_out[C:2 * C].rearrange("(i p) -> p i", p=128))
            m8 = misc.tile([128, 8], F32)
            va = misc.tile([128, 8], F32)
            vb = misc.tile([128, 8], F32)
            inv_n = 1.0 / (NCORES * T)
            nc.vector.tensor_scalar(out=vs_m[:, :], in0=vs_m[:, :],
                                    scalar1=inv_n, scalar2=None, op0=OP.mult)
            nc.vector.tensor_scalar(out=vs_e[:, :], in0=vs_e[:, :],
                                    scalar1=inv_n, scalar2=None, op0=OP.mult)
            nc.vector.tensor_tensor(out=m8[:, :], in0=vs_m[:, :], in1=vs_m[:, :], op=OP.mult)
            nc.vector.tensor_tensor(out=m8[:, :], in0=vs_e[:, :], in1=m8[:, :], op=OP.subtract)
            nc.scalar.activation(out=m8[:, :], in_=m8[:, :], func=ACTF.Sqrt,
                                 bias=eps_t[:, 0:1])
            nc.vector.reciprocal(out=m8[:, :], in_=m8[:, :])
            nc.vector.tensor_tensor(out=va[:, :], in0=m8[:, :], in1=gv8[:, :], op=OP.mult)
            nc.vector.tensor_tensor(out=vb[:, :], in0=vs_m[:, :], in1=va[:, :], op=OP.mult)
            nc.vector.tensor_tensor(out=vb[:, :], in0=bv8[:, :], in1=vb[:, :], op=OP.subtract)
            vb_b = misc.tile([128, 8], BF16)
            nc.vector.tensor_copy(out=vb_b[:, :], in_=vb[:, :])

            # wcvb[f] = sum_c Wc^T[c, f] * vb_c  (uses UNSCALED wc), then
            # roundtrip through DRAM to get it per-partition for the P7 drain
            wcvb_dram = nc.dram_tensor(f"wcvb_{rep}", [C], F32)
            wcvb_sb = misc.tile([1, C], F32)
            for fc in range(2):
                psw = psP.tile([1, 512], F32, tag=f"wcvb{fc}", bufs=1, name=f"wcvb{fc}")
                for ct in range(8):
                    nc.tensor.matmul(
                        psw[0:1, :],
                        vb_b[:, ct:ct + 1],
                        wc[ct][:, 512 * fc:512 * (fc + 1)],
                        start=(ct == 0), stop=(ct == 7))
                nc.vector.tensor_copy(out=wcvb_sb[0:1, 512 * fc:512 * (fc + 1)],
                                      in_=psw[0:1, :])
            nc.sync.dma_start(out=wcvb_dram[:], in_=wcvb_sb[0:1, :])
            wcvb_bc = misc.tile([128, 8], F32)
            nc.sync.dma_start(out=wcvb_bc[:, :],
                              in_=wcvb_dram[:].rearrange("(i p) -> p i", p=128))

            # scale wc rows by va (c is the partition dim of wc tiles)
            for ct in range(8):
                nc.vector.tensor_scalar(
                    out=wc[ct][:, :], in0=wc[ct][:, :],
                    scalar1=va[:, ct:ct + 1], scalar2=None, op0=OP.mult)

            # ---------------- P7: single projection, fused 1/r + bias ---------
            for tch in range(TQ):
                for ft in range(8):
                    ps = psA.tile([128, 512], F32, tag="mm", name=f"p2ps{tch}_{ft}")
                    for ct in range(8):
                        nc.tensor.matmul(
                            ps[:, :],
                            wc[ct][:, 128 * ft:128 * (ft + 1)],
                            y[ct][:, 512 * tch:512 * (tch + 1)],
                            start=(ct == 0), stop=(ct == 7))
                    o_t = outst.tile([128, 512], F32, tag="o", bufs=3,
                                     name=f"o{tch}_{ft}")
                    nc.vector.tensor_tensor(
                        out=o_t[:, :], in0=ps[:, :],
                        in1=r_bc[:, 512 * tch:512 * (tch + 1)], op=OP.mult)
                    nc.vector.tensor_scalar(
                        out=o_t[:, :], in0=o_t[:, :],
                        scalar1=wcvb_bc[:, ft:ft + 1], scalar2=None, op0=OP.add)
                    nc.sync.dma_start(
                        out=out[128 * ft:128 * (ft + 1), 512 * tch:512 * (tch + 1)],
                        in_=o_t[:, :])


_NC_CACHE = {}


def _get_nc(T):
    if T not in _NC_CACHE:
        _NC_CACHE[T] = build(T)
    return _NC_CACHE[T]


LAST_RESULTS = None
LAST_IN_MAPS = None


def make_in_maps(x, W_attn, W_proj, lora_attn_A, lora_attn_B, lora_proj_A,
                 lora_proj_B, bn_gamma, bn_beta, **_unused):
    f = np.float32
    bf = ml_dtypes.bfloat16
    x = np.asarray(x, f)
    B = x.shape[0]
    wT = np.ascontiguousarray(np.asarray(W_attn, f).T.astype(bf))      # [C, 3C]
    wp = np.asarray(W_proj, f)
    wpT = np.ascontiguousarray(wp.T.astype(bf))                        # [C, C]
    laT = np.ascontiguousarray(np.asarray(lora_attn_A, f).T.astype(bf))   # [R, C]
    lbB = np.ascontiguousarray(np.asarray(lora_attn_B, f).astype(bf))     # [R, 3C]
    lpaT = np.ascontiguousarray(np.asarray(lora_proj_A, f).T.astype(bf))  # [R, C]
    lpbB = np.ascontiguousarray(np.asarray(lora_proj_B, f).astype(bf))    # [R, C]
    gam = np.ascontiguousarray(np.asarray(bn_gamma, f))
    bet = np.ascontiguousarray(np.asarray(bn_beta, f))

    in_maps = []
    for b in range(B):
        in_maps.append({
            "xT": np.ascontiguousarray(x[b].T.astype(bf)),
            "wT": wT, "wpT": wpT,
            "wnat": np.ascontiguousarray(wp[:, 128 * b:128 * (b + 1)].astype(bf)),
            "laT": laT, "lbB": lbB,
            "lpaT": lpaT, "lpbB": lpbB, "gam": gam, "bet": bet,
        })
    return in_maps


def kernel(x, W_attn, W_proj, lora_attn_A, lora_attn_B, lora_proj_A, lora_proj_B,
           bn_gamma, bn_beta):
    global LAST_RESULTS, LAST_IN_MAPS
    f = np.float32
    x = np.asarray(x, f)
    B, T, C_ = x.shape
    assert C_ == C and B == NCORES

    in_maps = make_in_maps(x, W_attn, W_proj, lora_attn_A, lora_attn_B,
                           lora_proj_A, lora_proj_B, bn_gamma, bn_beta)
    LAST_IN_MAPS = in_maps
    nc = _get_nc(T)
    res = run_bass_kernel_spmd(nc, in_maps, core_ids=list(range(NCORES)))
    LAST_RESULTS = res
    return np.stack([np.asarray(res.results[b]["out"]).T for b in range(B)]).astype(f)
